# revision 19
# baseline (speedup 1.0000x reference)
"""Trainium2 Bass kernel for a dense transformer block (B=4,T=2048,H=16,D=64,C=1024,FF=4096).

Sharding: batch b -> core pair (2b, 2b+1). Within a pair, attention is split by
heads (8 heads/core, Megatron column-parallel QKV + row-parallel W_o), the
attention output partial sums are combined with a pair ReduceScatter, and each
core then runs the full-FF MLP on its half (1024) of the rows. Output rows are
disjoint across cores; the host just concatenates.

LayerNorm affines are folded into the following matmul weights on the host, so
the device only computes plain (x-mean)*rsqrt(var+eps). Matmuls run in bf16
with fp32 PSUM accumulation. Attention computes S^T = K @ Q^T directly so the
softmax probabilities are already in the [k, q] layout the AV matmul needs as
lhsT; the softmax denominator comes from a ones-column appended to V. The
causal mask is structural: fully-masked key blocks are never computed and the
diagonal blocks are multiplied by a constant 128x128 triangle after exp.
"""

import math

import ml_dtypes
import numpy as np

P = 128
B, T, H, D = 4, 2048, 16, 64
C = H * D
FF = 4096
EPS = 1e-5
N_CORES = 8

_CACHE = {}
LAST_RESULT = None


def _build(T, C, H, D, FF, n_cores, groups, phase_limit=99, sim_safe=False):
    """Build + compile the single-core SPMD program. Returns the Bacc object."""
    from contextlib import ExitStack

    import concourse.mybir as mybir
    import concourse.tile as tile
    from concourse import bacc

    dt = mybir.dt
    AF = mybir.ActivationFunctionType
    OP = mybir.AluOpType

    HH = H // 2               # heads per core
    QH = HH * D               # per-core c_out for each of q,k,v
    NQH = QH // P
    NT = T // P
    T2 = T // 2               # own rows
    NT2 = T2 // P
    NC = C // P
    NF = FF // P
    SL = min(512, T)          # attention q-slice width
    NSL = T // SL
    DBLK = SL // P
    HPC = P // D              # heads per 128-partition chunk
    FCW = min(512, FF)        # wfc col-chunk width
    TSW = min(512, T)         # qkv t-slice width
    NTS = T // TSW
    CSW = min(512, C)
    NCS = C // CSW
    TS2 = min(512, T2)
    NT2S = T2 // TS2
    NB = 4 if NC % 4 == 0 else 1  # transposes batched per psum bank
    assert QH % P == 0 and T % SL == 0 and SL % P == 0

    nc = bacc.Bacc("TRN2", target_bir_lowering=False, debug=False,
                   num_devices=n_cores)
    gelu_af = (mybir.ActivationFunctionType.Identity if sim_safe
               else mybir.ActivationFunctionType.Gelu)

    # ---- kernel I/O ----
    x_full = nc.dram_tensor("x_full", [T, C], dt.float32, kind="ExternalInput")
    x_own = nc.dram_tensor("x_own", [T2, C], dt.float32, kind="ExternalInput")
    wq = nc.dram_tensor("wq", [C, QH], dt.bfloat16, kind="ExternalInput")
    wk = nc.dram_tensor("wk", [C, QH], dt.bfloat16, kind="ExternalInput")
    wv = nc.dram_tensor("wv", [C, QH], dt.bfloat16, kind="ExternalInput")
    bq = nc.dram_tensor("bq", [QH], dt.float32, kind="ExternalInput")
    bk = nc.dram_tensor("bk", [QH], dt.float32, kind="ExternalInput")
    bv = nc.dram_tensor("bv", [QH], dt.float32, kind="ExternalInput")
    wo = nc.dram_tensor("wo", [QH, C], dt.bfloat16, kind="ExternalInput")
    bo = nc.dram_tensor("bo", [C], dt.float32, kind="ExternalInput")
    wfc = nc.dram_tensor("wfc", [C, FF], dt.bfloat16, kind="ExternalInput")
    bfc = nc.dram_tensor("bfc", [FF], dt.float32, kind="ExternalInput")
    wout = nc.dram_tensor("wout", [FF, C], dt.bfloat16, kind="ExternalInput")
    bout = nc.dram_tensor("bout", [C], dt.float32, kind="ExternalInput")
    tri = nc.dram_tensor("tri", [P, P], dt.bfloat16, kind="ExternalInput")
    ident = nc.dram_tensor("ident", [P, P], dt.bfloat16, kind="ExternalInput")
    out = nc.dram_tensor("out", [T2, C], dt.float32, kind="ExternalOutput")

    # collective bounce buffers (internal DRAM)
    r_bounce = nc.dram_tensor("r_bounce", [T, C], dt.float32)
    r_own_b = nc.dram_tensor("r_own_b", [T2, C], dt.float32)

    x_r = x_full.rearrange("(i p) c -> p i c", p=P)
    xo_r = x_own.rearrange("(i p) c -> p i c", p=P)
    out_r = out.rearrange("(i p) c -> p i c", p=P)
    rb_r = r_bounce.rearrange("(i p) c -> p i c", p=P)
    rob_r = r_own_b.rearrange("(i p) c -> p i c", p=P)

    with tile.TileContext(nc) as tc, ExitStack() as stk:
        pool_const = stk.enter_context(tc.tile_pool(name="const", bufs=1))

        tri_sb = pool_const.tile([P, P], dt.bfloat16)
        id_sb = pool_const.tile([P, P], dt.bfloat16)
        nc.sync.dma_start(tri_sb[:], tri[:])
        nc.sync.dma_start(id_sb[:], ident[:])
        bq_sb = pool_const.tile([P, NQH], dt.float32)
        bk_sb = pool_const.tile([P, NQH], dt.float32)
        bv_row = pool_const.tile([1, QH], dt.float32)
        bo_row = pool_const.tile([1, C], dt.float32)
        bfc_sb = pool_const.tile([P, NF], dt.float32)
        bout_row = pool_const.tile([1, C], dt.float32)
        eps_sb = pool_const.tile([P, 1], dt.float32)
        nc.vector.memset(eps_sb[:], EPS)
        ones1 = pool_const.tile([1, P], dt.float32)
        nc.vector.memset(ones1[:], 1.0)
        bv_full = pool_const.tile([P, QH], dt.float32)
        bo_full = pool_const.tile([P, C], dt.float32)
        bout_full = pool_const.tile([P, C], dt.float32)
        nc.sync.dma_start(bq_sb[:], bq.rearrange("(a p) -> p a", p=P))
        nc.sync.dma_start(bk_sb[:], bk.rearrange("(a p) -> p a", p=P))
        nc.sync.dma_start(bv_row[:], bv[None, :])
        nc.sync.dma_start(bo_row[:], bo[None, :])
        nc.sync.dma_start(bfc_sb[:], bfc.rearrange("(a p) -> p a", p=P))
        nc.sync.dma_start(bout_row[:], bout[None, :])

        def ln_finish(pool, s1, s2, n_chunks, nm):
            mean = pool.tile([P, n_chunks], dt.float32, tag="ln_mean",
                             name=f"mean_{nm}")
            var = pool.tile([P, n_chunks], dt.float32, tag="ln_var",
                            name=f"var_{nm}")
            rstd = pool.tile([P, n_chunks], dt.float32, tag="ln_rstd",
                             name=f"rstd_{nm}")
            nmr = pool.tile([P, n_chunks], dt.float32, tag="ln_nmr",
                            name=f"nmr_{nm}")
            nc.vector.tensor_scalar_mul(mean[:], s1[:], 1.0 / C)
            nc.vector.tensor_scalar_mul(var[:], s2[:], 1.0 / C)
            nc.vector.tensor_tensor(nmr[:], mean[:], mean[:], OP.mult)
            nc.vector.tensor_tensor(var[:], var[:], nmr[:], OP.subtract)
            nc.scalar.activation(var[:], var[:], AF.Sqrt, bias=eps_sb[:])
            nc.vector.reciprocal(rstd[:], var[:])
            nc.vector.tensor_tensor(nmr[:], mean[:], rstd[:], OP.mult)
            nc.vector.tensor_scalar_mul(nmr[:], nmr[:], -1.0)
            return rstd, nmr

        # attn persistents open first so everything transient frees above them
        with tc.tile_pool(name="pattn", bufs=1) as pool_attn:
            QT = pool_attn.tile([P, NQH, T], dt.bfloat16, tag="QT")
            KT = pool_attn.tile([P, NQH, T], dt.bfloat16, tag="KT")
            V = pool_attn.tile([P, NT, HH, D + 1], dt.bfloat16, tag="V")
            YT = pool_attn.tile([P, NQH, T], dt.bfloat16, tag="YT")
            wo_sb = pool_attn.tile([P, NQH, C], dt.bfloat16, tag="wo")

            with ExitStack() as es_zt:
                pool_zt = es_zt.enter_context(tc.tile_pool(name="pzt", bufs=1))
                ZT = pool_zt.tile([P, NC, T], dt.bfloat16)

                # ===== phase 0: stream x, LN1 stats, z, z^T =====
                with tc.tile_pool(name="pstat", bufs=1) as pool_stat, \
                     tc.tile_pool(name="pxs", bufs=3) as pool_xs, \
                     tc.tile_pool(name="ps_tra", bufs=2, space="PSUM") as ps_tra:
                    for row, full, w in ((bv_row, bv_full, QH),
                                         (bo_row, bo_full, C),
                                         (bout_row, bout_full, C)):
                        for o in range(0, w, 512):
                            wch = min(512, w - o)
                            pb = ps_tra.tile([P, 512], dt.float32, tag="bc")
                            nc.tensor.matmul(pb[:, :wch], ones1[:],
                                             row[:, o : o + wch])
                            nc.vector.tensor_copy(full[:, o : o + wch],
                                                  pb[:, :wch])
                    s1 = pool_stat.tile([P, NT], dt.float32, tag="s1")
                    s2 = pool_stat.tile([P, NT], dt.float32, tag="s2")
                    for i in range(NT):
                        xc = pool_xs.tile([P, C], dt.float32, tag="xc")
                        nc.sync.dma_start(xc[:], x_r[:, i, :])
                        nc.vector.reduce_sum(s1[:, i : i + 1], xc[:],
                                             axis=mybir.AxisListType.X)
                        sq = pool_xs.tile([P, C], dt.bfloat16, tag="sq")
                        nc.scalar.activation(sq[:], xc[:], AF.Square,
                                             accum_out=s2[:, i : i + 1])
                    rstd1, nmr1 = ln_finish(pool_stat, s1, s2, NT, "ln1")

                    for i in range(NT):
                        xc = pool_xs.tile([P, C], dt.float32, tag="xc")
                        nc.sync.dma_start(xc[:], x_r[:, i, :])
                        zc = pool_xs.tile([P, C], dt.bfloat16, tag="zc")
                        nc.scalar.activation(zc[:], xc[:], AF.Identity,
                                             bias=nmr1[:, i : i + 1],
                                             scale=rstd1[:, i : i + 1])
                        for jj in range(NC // NB):
                            pt = ps_tra.tile([P, NB * P], dt.bfloat16,
                                             tag="trp")
                            for j4 in range(NB):
                                j = jj * NB + j4
                                nc.tensor.transpose(
                                    pt[:, j4 * P : (j4 + 1) * P],
                                    zc[:, j * P : (j + 1) * P], id_sb[:])
                            nc.vector.tensor_copy(
                                ZT[:, jj * NB : (jj + 1) * NB,
                                   i * P : (i + 1) * P],
                                pt[:].rearrange("p (a b) -> p a b", a=NB))

                # ===== phase 1: QKV halves =====
                with tc.tile_pool(name="ps_mm1", bufs=4, space="PSUM") as ps_mm1:
                    with tc.tile_pool(name="pw1", bufs=1) as pool_wqkv:
                        wq_sb = pool_wqkv.tile([P, NC, QH], dt.bfloat16,
                                               tag="wq")
                        wk_sb = pool_wqkv.tile([P, NC, QH], dt.bfloat16,
                                               tag="wk")
                        wv_sb = pool_wqkv.tile([P, NC, QH], dt.bfloat16,
                                               tag="wv")
                        nc.sync.dma_start(
                            wq_sb[:], wq.rearrange("(ci p) o -> p ci o", p=P))
                        nc.sync.dma_start(
                            wk_sb[:], wk.rearrange("(ci p) o -> p ci o", p=P))
                        nc.sync.dma_start(
                            wv_sb[:], wv.rearrange("(ci p) o -> p ci o", p=P))

                        for w_sb, dstT, b_sb in ((wq_sb, QT, bq_sb),
                                                 (wk_sb, KT, bk_sb)):
                            for co in range(NQH):
                                for ts_ in range(NTS):
                                    pm = ps_mm1.tile([P, TSW], dt.float32,
                                                     tag="mmp")
                                    for ci in range(NC):
                                        nc.tensor.matmul(
                                            pm[:],
                                            w_sb[:, ci, co * P : (co + 1) * P],
                                            ZT[:, ci,
                                               ts_ * TSW : (ts_ + 1) * TSW],
                                            start=(ci == 0),
                                            stop=(ci == NC - 1))
                                    nc.scalar.activation(
                                        dstT[:, co, ts_ * TSW : (ts_ + 1) * TSW],
                                        pm[:], AF.Identity,
                                        bias=b_sb[:, co : co + 1])

                        for ti in range(NT):
                            pm = ps_mm1.tile([P, QH], dt.float32, tag="mmp")
                            for ci in range(NC):
                                nc.tensor.matmul(
                                    pm[:], ZT[:, ci, ti * P : (ti + 1) * P],
                                    wv_sb[:, ci, :],
                                    start=(ci == 0), stop=(ci == NC - 1))
                            nc.vector.tensor_tensor(
                                V[:, ti, :, :D],
                                pm[:].rearrange("p (h d) -> p h d", d=D),
                                bv_full[:].rearrange("p (h d) -> p h d", d=D),
                                OP.add)
                        nc.vector.memset(V[:, :, :, D], 1.0)

            # ===== phase 2: attention =====
            inv_sqrt_d = 1.0 / math.sqrt(D)
            with tc.tile_pool(name="ppt", bufs=2) as pool_pt, \
                 tc.tile_pool(name="prec", bufs=2) as pool_rec, \
                 tc.tile_pool(name="ps_s", bufs=2, space="PSUM") as ps_s, \
                 tc.tile_pool(name="ps_o", bufs=2, space="PSUM") as ps_o:
                for h in range(HH if phase_limit >= 2 else 0):
                    hc, hp = h // HPC, D * (h % HPC)
                    for s in range(NSL):
                        kc_max = (s + 1) * SL // P
                        PT = pool_pt.tile([P, NT, SL], dt.bfloat16, tag="PT")
                        for kc in range(kc_max):
                            # band blocks: columns left of the diagonal
                            # sub-block are fully masked -> zero, not exp'd
                            c0 = max(kc - s * DBLK, 0) * P
                            pm = ps_s.tile([P, SL], dt.float32, tag="sp")
                            nc.tensor.matmul(
                                pm[:, c0:],
                                KT[hp : hp + D, hc, kc * P : (kc + 1) * P],
                                QT[hp : hp + D, hc,
                                   s * SL + c0 : (s + 1) * SL],
                                start=True, stop=True)
                            nc.scalar.activation(PT[:, kc, c0:], pm[:, c0:],
                                                 AF.Exp, scale=inv_sqrt_d)
                            if c0 > 0:
                                nc.vector.memset(PT[:, kc, :c0], 0.0)
                        for j in range(DBLK):
                            kcd = s * DBLK + j
                            nc.vector.tensor_tensor(
                                PT[:, kcd, j * P : (j + 1) * P],
                                PT[:, kcd, j * P : (j + 1) * P],
                                tri_sb[:], OP.mult)
                        po = ps_o.tile([P, SL], dt.float32, tag="op")
                        for kc in range(kc_max):
                            nc.tensor.matmul(po[: D + 1, :], V[:, kc, h, :],
                                             PT[:, kc, :],
                                             start=(kc == 0),
                                             stop=(kc == kc_max - 1))
                        rec = pool_rec.tile([1, SL], dt.float32, tag="rec")
                        nc.vector.reciprocal(rec[:], po[D : D + 1, :])
                        pb = ps_s.tile([P, SL], dt.float32, tag="rb")
                        nc.tensor.matmul(pb[:D, :], ones1[:, :D], rec[:])
                        recb = pool_rec.tile([D, SL], dt.float32, tag="recb")
                        nc.vector.tensor_copy(recb[:], pb[:D, :])
                        nc.vector.tensor_tensor(
                            YT[hp : hp + D, hc, s * SL : (s + 1) * SL],
                            po[:D, :], recb[:], OP.mult)

            # ===== phase 3: W_o partial -> r_bounce =====
            nc.sync.dma_start(wo_sb[:],
                              wo.rearrange("(ci p) o -> p ci o", p=P))
            with tc.tile_pool(name="prs", bufs=3) as pool_rs, \
                 tc.tile_pool(name="ps_mm3", bufs=4, space="PSUM") as ps_mm3:
                for ti in range(NT if phase_limit >= 3 else 0):
                    r_sb = pool_rs.tile([P, C], dt.float32, tag="rsb")
                    for cs in range(NCS):
                        pm = ps_mm3.tile([P, CSW], dt.float32, tag="mmp")
                        for ci in range(NQH):
                            nc.tensor.matmul(
                                pm[:],
                                YT[:, ci, ti * P : (ti + 1) * P],
                                wo_sb[:, ci, cs * CSW : (cs + 1) * CSW],
                                start=(ci == 0), stop=(ci == NQH - 1))
                        nc.vector.tensor_copy(
                            r_sb[:, cs * CSW : (cs + 1) * CSW], pm[:])
                    nc.sync.dma_start(rb_r[:, ti, :], r_sb[:])

        # ===== ReduceScatter over the pair =====
        if phase_limit >= 4:
            nc.gpsimd.collective_compute(
                "ReduceScatter", OP.add, replica_groups=groups,
                ins=[r_bounce.ap().opt()], outs=[r_own_b.ap().opt()])

        # ===== phase 4 + 5 =====
        with tc.tile_pool(name="px2", bufs=1) as pool_x2:
            X2 = pool_x2.tile([P, NT2, C], dt.float32, tag="x2")
            s1b = pool_x2.tile([P, NT2], dt.float32, tag="s1b")
            s2b = pool_x2.tile([P, NT2], dt.float32, tag="s2b")

            with tc.tile_pool(name="pht", bufs=1) as pool_ht:
                HT = pool_ht.tile([P, NF, T2], dt.bfloat16)

                with ExitStack() as es_z2t:
                    pool_z2t = es_z2t.enter_context(
                        tc.tile_pool(name="pz2t", bufs=1))
                    Z2T = pool_z2t.tile([P, NC, T2], dt.bfloat16)

                    # phase 4: residual + LN2 + z2 + z2^T
                    with tc.tile_pool(name="pxo", bufs=3) as pool_xo, \
                         tc.tile_pool(name="ps_trb", bufs=2,
                                      space="PSUM") as ps_trb:
                        NT2_g = NT2 if phase_limit >= 5 else 0
                        for i in range(NT2_g):
                            xoc = pool_xo.tile([P, C], dt.float32, tag="xoc")
                            roc = pool_xo.tile([P, C], dt.float32, tag="roc")
                            nc.sync.dma_start(xoc[:], xo_r[:, i, :])
                            nc.sync.dma_start(roc[:], rob_r[:, i, :])
                            nc.vector.tensor_tensor(X2[:, i, :], xoc[:],
                                                    roc[:], OP.add)
                            nc.vector.tensor_tensor(
                                X2[:, i, :], X2[:, i, :], bo_full[:], OP.add)
                            nc.vector.reduce_sum(s1b[:, i : i + 1],
                                                 X2[:, i, :],
                                                 axis=mybir.AxisListType.X)
                            sq = pool_xo.tile([P, C], dt.bfloat16, tag="sq2")
                            nc.scalar.activation(sq[:], X2[:, i, :],
                                                 AF.Square,
                                                 accum_out=s2b[:, i : i + 1])
                        if NT2_g:
                            rstd2, nmr2 = ln_finish(pool_x2, s1b, s2b, NT2,
                                                    "ln2")
                        for i in range(NT2_g):
                            z2c = pool_xo.tile([P, C], dt.bfloat16, tag="z2c")
                            nc.scalar.activation(z2c[:], X2[:, i, :],
                                                 AF.Identity,
                                                 bias=nmr2[:, i : i + 1],
                                                 scale=rstd2[:, i : i + 1])
                            for jj in range(NC // NB):
                                pt = ps_trb.tile([P, NB * P], dt.bfloat16,
                                                 tag="trp")
                                for j4 in range(NB):
                                    j = jj * NB + j4
                                    nc.tensor.transpose(
                                        pt[:, j4 * P : (j4 + 1) * P],
                                        z2c[:, j * P : (j + 1) * P], id_sb[:])
                                nc.vector.tensor_copy(
                                    Z2T[:, jj * NB : (jj + 1) * NB,
                                        i * P : (i + 1) * P],
                                    pt[:].rearrange("p (a b) -> p a b", a=NB))

                    # phase 5a: FC + gelu
                    with tc.tile_pool(name="pwfc", bufs=2) as pool_wfc, \
                         tc.tile_pool(name="ps_h", bufs=4,
                                      space="PSUM") as ps_h:
                        for fo in range(FF // FCW if phase_limit >= 6 else 0):
                            wfc_sb = pool_wfc.tile([P, NC, FCW], dt.bfloat16,
                                                   tag="wfc")
                            nc.sync.dma_start(
                                wfc_sb[:],
                                wfc[:, fo * FCW : (fo + 1) * FCW]
                                .rearrange("(ci p) o -> p ci o", p=P))
                            for f in range(FCW // P):
                                fg = fo * (FCW // P) + f
                                for ts_ in range(NT2S):
                                    pm = ps_h.tile([P, TS2], dt.float32,
                                                   tag="hp")
                                    for ci in range(NC):
                                        nc.tensor.matmul(
                                            pm[:],
                                            wfc_sb[:, ci, f * P : (f + 1) * P],
                                            Z2T[:, ci,
                                                ts_ * TS2 : (ts_ + 1) * TS2],
                                            start=(ci == 0),
                                            stop=(ci == NC - 1))
                                    nc.scalar.activation(
                                        HT[:, fg, ts_ * TS2 : (ts_ + 1) * TS2],
                                        pm[:], gelu_af,
                                        bias=bfc_sb[:, fg : fg + 1])

                # phase 5b: W_out + residual
                with tc.tile_pool(name="pwout", bufs=3) as pool_wout, \
                     tc.tile_pool(name="pout", bufs=3) as pool_out, \
                     tc.tile_pool(name="ps_out", bufs=1,
                                  space="PSUM") as ps_out:
                    for cs in range(NCS if phase_limit >= 7 else 0):
                        pms = [ps_out.tile([P, CSW], dt.float32,
                                           tag=f"outp{ti}",
                                           name=f"outp_{cs}_{ti}")
                               for ti in range(NT2)]
                        for fi in range(NF):
                            wout_sb = pool_wout.tile([P, CSW], dt.bfloat16,
                                                     tag="wout")
                            nc.sync.dma_start(
                                wout_sb[:],
                                wout[fi * P : (fi + 1) * P,
                                     cs * CSW : (cs + 1) * CSW])
                            for ti in range(NT2):
                                nc.tensor.matmul(
                                    pms[ti][:],
                                    HT[:, fi, ti * P : (ti + 1) * P],
                                    wout_sb[:],
                                    start=(fi == 0), stop=(fi == NF - 1))
                        for ti in range(NT2):
                            o_sb = pool_out.tile([P, CSW], dt.float32,
                                                 tag="osb")
                            nc.vector.tensor_tensor(
                                o_sb[:], pms[ti][:],
                                X2[:, ti, cs * CSW : (cs + 1) * CSW], OP.add)
                            nc.vector.tensor_tensor(
                                o_sb[:], o_sb[:],
                                bout_full[:, cs * CSW : (cs + 1) * CSW],
                                OP.add)
                            nc.sync.dma_start(
                                out_r[:, ti, cs * CSW : (cs + 1) * CSW],
                                o_sb[:])

    nc.compile()
    return nc


def _prep_core_inputs(b, parity, x, ln1_w, ln1_b, w_qkv, b_qkv, w_o, b_o,
                      ln2_w, ln2_b, w_fc, b_fc, w_out, b_out,
                      T_, C_, H_, D_):
    """Host-side per-core input dict (weights LN-folded, matmul inputs bf16)."""
    bf16 = ml_dtypes.bfloat16
    HH = H_ // 2
    QH = HH * D_
    T2 = T_ // 2
    wq_eff = (ln1_w[:, None] * w_qkv).astype(np.float32)
    bq_eff = (b_qkv + ln1_b @ w_qkv).astype(np.float32)
    wfc_eff = (ln2_w[:, None] * w_fc).astype(np.float32)
    bfc_eff = (b_fc + ln2_b @ w_fc).astype(np.float32)

    h0 = parity * QH
    sl_q = slice(h0, h0 + QH)
    sl_k = slice(C_ + h0, C_ + h0 + QH)
    sl_v = slice(2 * C_ + h0, 2 * C_ + h0 + QH)
    tri = np.tril(np.ones((P, P), np.float32)).T  # tri[k,q] = 1 if k <= q
    ident = np.eye(P, dtype=np.float32)
    return {
        "x_full": np.ascontiguousarray(x[b]),
        "x_own": np.ascontiguousarray(x[b, parity * T2 : (parity + 1) * T2]),
        "wq": np.ascontiguousarray(wq_eff[:, sl_q]).astype(bf16),
        "wk": np.ascontiguousarray(wq_eff[:, sl_k]).astype(bf16),
        "wv": np.ascontiguousarray(wq_eff[:, sl_v]).astype(bf16),
        "bq": np.ascontiguousarray(bq_eff[sl_q]),
        "bk": np.ascontiguousarray(bq_eff[sl_k]),
        "bv": np.ascontiguousarray(bq_eff[sl_v]),
        "wo": np.ascontiguousarray(w_o[h0 : h0 + QH, :]).astype(bf16),
        "bo": np.ascontiguousarray(b_o),
        "wfc": np.ascontiguousarray(wfc_eff).astype(bf16),
        "bfc": np.ascontiguousarray(bfc_eff),
        "wout": np.ascontiguousarray(w_out).astype(bf16),
        "bout": np.ascontiguousarray(b_out),
        "tri": tri.astype(bf16),
        "ident": ident.astype(bf16),
    }


def kernel(x, ln1_w, ln1_b, w_qkv, b_qkv, w_o, b_o, ln2_w, ln2_b,
           w_fc, b_fc, w_out, b_out):
    from concourse.bass_utils import run_bass_kernel_spmd

    key = (T, C, H, D, FF, N_CORES)
    if key not in _CACHE:
        groups = [[2 * i, 2 * i + 1] for i in range(N_CORES // 2)]
        _CACHE[key] = _build(T, C, H, D, FF, N_CORES, groups)
    nc = _CACHE[key]

    args = (np.asarray(x, np.float32), np.asarray(ln1_w, np.float32),
            np.asarray(ln1_b, np.float32), np.asarray(w_qkv, np.float32),
            np.asarray(b_qkv, np.float32), np.asarray(w_o, np.float32),
            np.asarray(b_o, np.float32), np.asarray(ln2_w, np.float32),
            np.asarray(ln2_b, np.float32), np.asarray(w_fc, np.float32),
            np.asarray(b_fc, np.float32), np.asarray(w_out, np.float32),
            np.asarray(b_out, np.float32))
    in_maps = []
    for core in range(N_CORES):
        b, parity = core // 2, core % 2
        in_maps.append(_prep_core_inputs(b, parity, *args, T, C, H, D))

    global LAST_RESULT
    res = run_bass_kernel_spmd(nc, in_maps, core_ids=list(range(N_CORES)))
    LAST_RESULT = res

    T2 = T // 2
    full = np.empty((B, T, C), np.float32)
    for core in range(N_CORES):
        b, parity = core // 2, core % 2
        full[b, parity * T2 : (parity + 1) * T2] = res.results[core]["out"]
    return full
